# revision 16
# baseline (speedup 1.0000x reference)
"""Trainium2 Bass kernel for nn_DecoderAttn (B=32, T=128, L=2048, D=F=1024).

Strategy (v2)
-------------
Data-parallel over batch: 4 batches per NeuronCore x 8 cores, no collectives.

Same algebraic restructure as v1 (scores via kq = (W_k.T proj_q)/32, softmax
without max-subtraction, 1/denom folded into the epilogue), plus:

1. RNN stall kill: xw_t is pre-written into the step's PSUM banks by the DVE
   (2 copies/step), the W_hh matmuls accumulate onto it (start=False), and
   tanh reads PSUM directly.  This removes the per-step add->tanh DVE hop
   that stalled the PE 666 ns at every step boundary (measured); the RNN
   runs at the LDWEIGHTS floor (64 x 27 ns/step).
2. hs preload: ALL attention hs data that fits is DMA'd into SBUF during the
   RNN (DMA is otherwise idle for ~230 us).  NN l-tiles stay natural
   (DVE-scores + PE-context), NT l-tiles are preloaded transposed
   (PE-scores); their natural twins stream post-RNN for context, rotating
   through the same SBUF slots (tag-based WAR).
3. kq broadcast without DMA hops: kqt columns are PE-transposed to a row at
   partition 0, then K=1-broadcast across partitions.
4. Epilogue in f-major (outT) layout: W_reg tiles are the stationary
   (LDW-bound, 128x27ns) instead of 512-column moving streams; b_reg folds
   in via K=1 matmuls; 1/denom applied per-column via a broadcast tile.
   Host transposes the [F, BL] result.

All matmul operands fp16 (PSUM accumulates fp32); fp8 was measured and
rejected (LDWEIGHTS is element-rate bound so fp8 loads no faster, DoubleRow
LDW is 4.7x slower, and fp8 W_hh amplifies to 1.8e-2 output error through
the 128-step recurrence).
"""

import sys
from contextlib import ExitStack

for _p in ("/opt/trn_rl_repo",):
    if _p not in sys.path:
        sys.path.insert(0, _p)

import numpy as np

import concourse.bass as bass
import concourse.mybir as mybir
from concourse.tile import TileContext

AF = mybir.ActivationFunctionType
ALU = mybir.AluOpType
f16 = mybir.dt.float16
f32 = mybir.dt.float32


def _split_multiwaits(nc):
    """Walrus in this environment rejects >1 sync-wait per compute
    instruction ("Too many sync wait commands"). Split extras into
    preceding single-wait EventSemaphore instructions on the same engine
    (the same encoding raw-bass wait_ge() uses) — semantically identical
    since engine streams execute in order."""
    for f in nc.m.functions:
        for blk in f.blocks:
            new = []
            for inst in blk.instructions:
                si = inst.sync_info
                if si is not None and si.on_wait is not None and len(si.on_wait) > 1:
                    for j, w in enumerate(list(si.on_wait)[:-1]):
                        es = mybir.InstEventSemaphore(
                            name=f"{inst.name}-mw{j}", ins=[], outs=[])
                        es.engine = inst.engine
                        es.debug = inst.debug
                        es.sync_info = mybir.SyncInfo(on_wait=[w], on_update=[])
                        new.append(es)
                    inst.sync_info = mybir.SyncInfo(
                        on_wait=[si.on_wait[-1]], on_update=si.on_update)
                new.append(inst)
            blk.instructions[:] = new
    return nc


P = 128          # partitions
BL = 4           # batches per core
NCORES = 8
T = 128          # decoder steps
L = 2048         # encoder length
D = 1024         # hidden dim
F = 1024         # n_features
ND = D // P      # 8 d/e/f tiles
NH = ND // 2     # 4 tiles per ek-half
NL = L // P      # 16 l tiles
NC = (2 * D) // P  # 16 concat tiles
TB = T * BL      # 512 (t,b) columns
NN = 8           # l-tiles preloaded natural (DVE scores, PE context)
NT = NL - NN     # l-tiles preloaded transposed (PE scores; natural streamed)


def build_program(split=True, debug=False):
    nc = bass.Bass()
    if debug:
        dbgh_d = nc.declare_dram_parameter("dbg_h", [P, 2 * NH * BL], f16, isOutput=True)
        dbgs_d = nc.declare_dram_parameter("dbg_s", [BL, P, NL], f32, isOutput=True)
        dbgc_d = nc.declare_dram_parameter("dbg_c", [P, NC * BL], f16, isOutput=True)

    # ---- I/O ----
    xT_d = nc.declare_dram_parameter("xT16", [D, TB], f16, isOutput=False)
    wih_d = nc.declare_dram_parameter("wihT16", [D, D], f16, isOutput=False)
    whh_d = nc.declare_dram_parameter("whhT16", [D, D], f16, isOutput=False)
    wq_d = nc.declare_dram_parameter("wqT16", [D, D], f16, isOutput=False)
    wk_d = nc.declare_dram_parameter("wk16", [D, D], f16, isOutput=False)
    wreg_d = nc.declare_dram_parameter("wregT16", [2 * D, F], f16, isOutput=False)
    h0_d = nc.declare_dram_parameter("h0T16", [D, BL], f16, isOutput=False)
    bihh_d = nc.declare_dram_parameter("bihh", [D, 1], f32, isOutput=False)
    bq_d = nc.declare_dram_parameter("bq", [D, 1], f32, isOutput=False)
    breg_d = nc.declare_dram_parameter("breg1", [1, F], f16, isOutput=False)
    ident_d = nc.declare_dram_parameter("ident16", [P, P], f16, isOutput=False)
    hs_d = nc.declare_dram_parameter("hs16", [BL, L, D], f16, isOutput=False)
    hsTh_d = nc.declare_dram_parameter("hsTh16", [BL, D, NT * P], f16, isOutput=False)
    out_d = nc.declare_dram_parameter("outT", [F, BL], f32, isOutput=True)

    with TileContext(nc) as tc, ExitStack() as stack:
        const = stack.enter_context(tc.tile_pool(name="const", bufs=1))

        # ---- persistent SBUF tiles ----
        xwA = const.tile([P, T, 4 * BL], f16, name="xwA")
        xwB = const.tile([P, T, 4 * BL], f16, name="xwB")
        hA = [const.tile([P, NH * BL], f16, name=f"hA_{p}") for p in range(2)]
        hB = [const.tile([P, NH * BL], f16, name=f"hB_{p}") for p in range(2)]
        bihh_t = [const.tile([P, 1], f32, name=f"bihh_{k}") for k in range(ND)]
        bq_t = [const.tile([P, 1], f32, name=f"bq_{k}") for k in range(ND)]
        pq = [const.tile([P, BL], f16, name=f"pq_{k}") for k in range(ND)]
        kqt = [const.tile([P, BL], f16, name=f"kqt_{k}") for k in range(ND)]
        krow1 = const.tile([1, D], f16, name="krow1")
        scores_sb = [const.tile([P, NL], f32, name=f"ssb_{b}") for b in range(BL)]
        p16 = [const.tile([P, NL], f16, name=f"p16_{b}") for b in range(BL)]
        accall = const.tile([P, BL], f32, name="accall")
        acc16all = const.tile([P, BL], f16, name="acc16all")
        rec_row = const.tile([1, BL], f32, name="rec_row")
        rec16_row = const.tile([1, BL], f16, name="rec16_row")
        rec_bc = const.tile([P, BL], f32, name="rec_bc")
        concat = const.tile([P, NC, BL], f16, name="concat")
        tmp_ep = const.tile([P, ND, BL], f32, name="tmp_ep")
        out_sbT = const.tile([P, ND, BL], f32, name="out_sbT")
        ones_col = const.tile([P, 1], f16, name="ones_col")
        ones_row = const.tile([1, P], f16, name="ones_row")
        breg_t = const.tile([1, F], f16, name="breg_t")
        ident = const.tile([P, P], f16, name="ident")

        # natural hs preload (lives to the end)
        natp = stack.enter_context(tc.tile_pool(name="natp", bufs=1))
        nat = [[natp.tile([P, D], f16, name=f"nat_{b}_{lt}")
                for lt in range(NN)] for b in range(BL)]
        # transposed hs preload; the post-RNN natural twins rotate through
        # the same slots (same tag, bufs=1 -> automatic WAR on the readers)
        attnp = [stack.enter_context(tc.tile_pool(name=f"attnp{b}", bufs=1))
                 for b in range(BL)]
        hsT = [[attnp[b].tile([P, D], f16, name=f"hsT_{b}_{ek}", tag=f"s{ek}")
                for ek in range(ND)] for b in range(BL)]

        # early-dying weight pools on the right side (LIFO: ph1 first, then
        # whh at RNN end, then wq/wk after phase 3)
        pwqk = tc.tile_pool(name="pwqk", bufs=1, side="right")
        pwqk_pool = pwqk.__enter__()
        wq = [pwqk_pool.tile([P, D], f16, name=f"wq_{k}") for k in range(ND)]
        wk = [pwqk_pool.tile([P, D], f16, name=f"wk_{k}") for k in range(ND)]
        pwhh = tc.tile_pool(name="pwhh", bufs=1, side="right")
        pwhh_pool = pwhh.__enter__()
        whh = [pwhh_pool.tile([P, D], f16, name=f"whh_{k}") for k in range(ND)]

        with tc.tile_pool(name="ph1", bufs=4, side="right") as ph1:
            # wih/xT rotate through 4 slots; fk 0..3 DMA wait-free first so
            # the later weights aren't queued behind WAR-blocked transfers.
            wih = [None] * ND
            xT = [None] * ND

            # ---- input DMAs, critical-path first (program order = FIFO
            # priority within each DMA queue) ----
            for fk in range(4):
                wih[fk] = ph1.tile([P, D], f16, name=f"wih_{fk}", tag="wih")
                xT[fk] = ph1.tile([P, TB], f16, name=f"xT_{fk}", tag="xT")
                nc.sync.dma_start(wih[fk][:], wih_d[fk * P:(fk + 1) * P, :])
                nc.sync.dma_start(xT[fk][:], xT_d[fk * P:(fk + 1) * P, :])
            for k in range(ND):
                nc.sync.dma_start(bihh_t[k][:], bihh_d[k * P:(k + 1) * P, :])
            for k in range(ND):
                nc.sync.dma_start(whh[k][:], whh_d[k * P:(k + 1) * P, :])
            for k in range(ND):
                half = hA[0] if k < NH else hB[0]
                j = k % NH
                nc.sync.dma_start(half[:, j * BL:(j + 1) * BL], h0_d[k * P:(k + 1) * P, :])
            for k in range(ND):
                nc.sync.dma_start(wq[k][:], wq_d[k * P:(k + 1) * P, :])
                nc.sync.dma_start(wk[k][:], wk_d[k * P:(k + 1) * P, :])
                nc.sync.dma_start(bq_t[k][:], bq_d[k * P:(k + 1) * P, :])
            nc.sync.dma_start(breg_t[:], breg_d[:])
            nc.sync.dma_start(ident[:], ident_d[:])
            nc.any.memset(ones_col[:], 1.0)
            nc.any.memset(ones_row[:], 1.0)
            for fk in range(4, ND):
                wih[fk] = ph1.tile([P, D], f16, name=f"wih_{fk}", tag="wih")
                xT[fk] = ph1.tile([P, TB], f16, name=f"xT_{fk}", tag="xT")
                nc.sync.dma_start(wih[fk][:], wih_d[fk * P:(fk + 1) * P, :])
                nc.sync.dma_start(xT[fk][:], xT_d[fk * P:(fk + 1) * P, :])
            # hs preload (the bulk; queued behind the weights)
            for b in range(BL):
                for lt in range(NN):
                    nc.sync.dma_start(nat[b][lt][:], hs_d[b, lt * P:(lt + 1) * P, :])
            for b in range(BL):
                for ek in range(ND):
                    nc.sync.dma_start(hsT[b][ek][:], hsTh_d[b, ek * P:(ek + 1) * P, :])

            # ---- phase 1: xw = W_ih @ X.T + (b_ih + b_hh), written f16 in
            # step-contiguous [P, T, (dt%4)*4+b] halves ----
            with tc.tile_pool(name="psx", bufs=1, space="PSUM") as psx:
                ps_x = [psx.tile([P, T, BL], f32, name=f"ps_x{k}", tag=f"psx{k}")
                        for k in range(ND)]
                for fk in range(ND):
                    for dt in range(ND):
                        nc.tensor.matmul(
                            ps_x[dt][:], wih[fk][:, dt * P:(dt + 1) * P], xT[fk][:],
                            start=(fk == 0), stop=(fk == ND - 1))
                for dt in range(ND):
                    xw_half = xwA if dt < NH else xwB
                    j = dt % NH
                    nc.scalar.activation(
                        xw_half[:, :, j * BL:(j + 1) * BL], ps_x[dt][:],
                        AF.Identity, bias=bihh_t[dt][:])
        # ph1 closed

        # ---- phase 2: RNN, 128 steps at the LDWEIGHTS floor ----
        with tc.tile_pool(name="psh", bufs=2, space="PSUM") as psh:
            cur, nxt = (hA[0], hB[0]), (hA[1], hB[1])
            for t in range(T):
                psA = psh.tile([P, NH * BL], f32, name="psA", tag="psA")
                psB = psh.tile([P, NH * BL], f32, name="psB", tag="psB")
                # pre-write xw_t into PSUM (off the critical path)
                nc.vector.tensor_copy(psA[:], xwA[:, t, :])
                nc.vector.tensor_copy(psB[:], xwB[:, t, :])
                # pass 1: contract ek-half A for all dt
                for dt in range(ND):
                    ps = psA if dt < NH else psB
                    j = dt % NH
                    for ek in range(NH):
                        nc.tensor.matmul(
                            ps[:, j * BL:(j + 1) * BL],
                            whh[ek][:, dt * P:(dt + 1) * P],
                            cur[0][:, ek * BL:(ek + 1) * BL],
                            start=False, stop=False)
                # pass 2: contract ek-half B; psA's groups close first
                for dt in range(ND):
                    ps = psA if dt < NH else psB
                    j = dt % NH
                    for ek in range(NH, ND):
                        nc.tensor.matmul(
                            ps[:, j * BL:(j + 1) * BL],
                            whh[ek][:, dt * P:(dt + 1) * P],
                            cur[1][:, (ek - NH) * BL:(ek - NH + 1) * BL],
                            start=False, stop=(ek == ND - 1))
                # tanh straight from PSUM; half A feeds next step's pass 1
                nc.scalar.activation(nxt[0][:], psA[:], AF.Tanh)
                nc.scalar.activation(nxt[1][:], psB[:], AF.Tanh)
                cur, nxt = nxt, cur
        # final hidden state (query) lives in `cur` (A, B halves)
        pwhh.__exit__(None, None, None)
        if debug:
            nc.sync.dma_start(dbgh_d[:, 0:NH * BL], cur[0][:])
            nc.sync.dma_start(dbgh_d[:, NH * BL:], cur[1][:])

        # copy query into concat tiles ct 8..15 (layout [P, ct, b])
        nc.vector.tensor_copy(concat[:, ND:ND + NH, :], cur[0][:])
        nc.vector.tensor_copy(concat[:, ND + NH:NC, :], cur[1][:])

        # ---- phase 3: proj_q; kqt columns (scaled 1/32) ----
        with tc.tile_pool(name="psq", bufs=2, space="PSUM") as psq:
            for dt in range(ND):
                ps = psq.tile([P, BL], f32, name="ps_q", tag="psq")
                for dk in range(ND):
                    half = cur[0] if dk < NH else cur[1]
                    j = dk % NH
                    nc.tensor.matmul(
                        ps[:], wq[dk][:, dt * P:(dt + 1) * P],
                        half[:, j * BL:(j + 1) * BL],
                        start=(dk == 0), stop=(dk == ND - 1))
                nc.scalar.activation(pq[dt][:], ps[:], AF.Identity, bias=bq_t[dt][:])
            for et in range(ND):
                ps = psq.tile([P, BL], f32, name="ps_kt", tag="psq")
                for dk in range(ND):
                    nc.tensor.matmul(
                        ps[:], wk[dk][:, et * P:(et + 1) * P], pq[dk][:],
                        start=(dk == 0), stop=(dk == ND - 1))
                nc.vector.tensor_scalar_mul(kqt[et][:], ps[:], 1.0 / 32.0)
        pwqk.__exit__(None, None, None)
        kqbp_cm = tc.tile_pool(name="kqbp", bufs=1, side="right")
        kqbp = kqbp_cm.__enter__()
        kqb16 = [kqbp.tile([P, D], f16, name=f"kqb16_{b}") for b in range(BL)]

        # ---- phases 4-6 per batch ----
        nat2 = [None] * BL
        with tc.tile_pool(name="psrow", bufs=2, space="PSUM") as psrow, \
             tc.tile_pool(name="psbc", bufs=2, space="PSUM") as psbc, \
             tc.tile_pool(name="pss", bufs=2, space="PSUM") as pss, \
             tc.tile_pool(name="psc", bufs=2, space="PSUM") as pscp, \
             tc.tile_pool(name="scrp", bufs=2) as scrp:
            for b in range(BL):
                # kq broadcast across partitions: transpose kqt columns to a
                # row at partition 0, then K=1 ones-matmul broadcast.
                ps_row = psrow.tile([1, D], f16, name="ps_row", tag="psrow")
                for et in range(ND):
                    nc.tensor.transpose(
                        ps_row[0:1, et * P:(et + 1) * P], kqt[et][:, b:b + 1],
                        ident[:])
                nc.scalar.activation(krow1[:], ps_row[:], AF.Copy)
                for h in range(2):
                    ps_bc = psbc.tile([P, 512], f32, name="ps_bc", tag="psbc")
                    nc.tensor.matmul(ps_bc[:], ones_row[:],
                                     krow1[0:1, h * 512:(h + 1) * 512],
                                     start=True, stop=True)
                    nc.scalar.activation(
                        kqb16[b][:, h * 512:(h + 1) * 512], ps_bc[:], AF.Copy)
                # PE scores for the NT transposed tiles
                ps_s = pss.tile([P, NT], f32, name="ps_s", tag="pss")
                for j in range(NT):
                    for ek in range(ND):
                        nc.tensor.matmul(
                            ps_s[:, j:j + 1],
                            hsT[b][ek][:, j * P:(j + 1) * P],
                            kqt[ek][:, b:b + 1],
                            start=(ek == 0), stop=(ek == ND - 1))
                nc.vector.tensor_copy(scores_sb[b][:, NN:NL], ps_s[:])
                # stream this b's natural twins into the hsT slots (tag WAR:
                # each DMA waits only on that slot's PE-score reads)
                nat2[b] = [attnp[b].tile([P, D], f16, name=f"nat2_{b}_{j}",
                                         tag=f"s{j}")
                           for j in range(NT)]
                for j in range(NT):
                    nc.sync.dma_start(nat2[b][j][:],
                                      hs_d[b, (NN + j) * P:(NN + j + 1) * P, :])
                # DVE scores for the NN natural tiles (accum straight into
                # the scores column)
                for lt in range(NN):
                    scr = scrp.tile([P, D], f16, name="scr", tag="scr")
                    nc.vector.scalar_tensor_tensor(
                        scr[:], nat[b][lt][:], 1.0, kqb16[b][:],
                        op0=ALU.mult, op1=ALU.mult,
                        accum_out=scores_sb[b][:, lt:lt + 1])
                nc.scalar.activation(p16[b][:], scores_sb[b][:], AF.Exp,
                                     accum_out=accall[:, b:b + 1])
                # context: 16 lt x 8 et accumulating matmuls.  The et groups
                # interleave within one PSUM bank, so no start=True is allowed
                # (its pending-zero marking clobbers sibling columns) — the
                # bank is zeroed by the DVE instead, like the RNN pre-write.
                ps_c = pscp.tile([P, ND], f32, name="ps_c", tag="psc")
                nc.vector.memset(ps_c[:], 0.0)
                for i in range(NL):
                    src = nat[b][i] if i < NN else nat2[b][i - NN]
                    for et in range(ND):
                        nc.tensor.matmul(
                            ps_c[:, et:et + 1], src[:, et * P:(et + 1) * P],
                            p16[b][:, i:i + 1],
                            start=False, stop=(i == NL - 1))
                nc.vector.tensor_copy(concat[:, 0:ND, b], ps_c[:])
                if debug:
                    nc.sync.dma_start(dbgs_d[b, :, :], scores_sb[b][:])
        if debug:
            nc.sync.dma_start(dbgc_d[:], concat[:])

        # ---- epilogue: outT[f, b] = ctx_unnorm*rec + (q @ W_reg + breg) ----
        with tc.tile_pool(name="pse", bufs=1, space="PSUM") as psep, \
             tc.tile_pool(name="wrgp", bufs=4) as wrgp:
            nc.vector.tensor_copy(acc16all[:], accall[:])
            ps_dr = psep.tile([1, BL], f32, name="ps_dr", tag="psdr")
            nc.tensor.matmul(ps_dr[:], ones_col[:], acc16all[:],
                             start=True, stop=True)
            nc.vector.reciprocal(rec_row[:], ps_dr[:])
            nc.vector.tensor_copy(rec16_row[:], rec_row[:])
            ps_rb = psep.tile([P, BL], f32, name="ps_rb", tag="psrb")
            nc.tensor.matmul(ps_rb[:], ones_row[:], rec16_row[:],
                             start=True, stop=True)
            nc.vector.tensor_copy(rec_bc[:], ps_rb[:])
            ps_ctx = psep.tile([P, ND, BL], f32, name="ps_ctx", tag="psectx")
            ps_q2 = psep.tile([P, ND, BL], f32, name="ps_q2", tag="pseq")
            nc.vector.memset(ps_ctx[:], 0.0)
            nc.vector.memset(ps_q2[:], 0.0)
            for ct in range(NC):
                wrg = wrgp.tile([P, F], f16, name="wrg", tag="wrg")
                nc.sync.dma_start(wrg[:], wreg_d[ct * P:(ct + 1) * P, :])
                dst = ps_ctx if ct < ND else ps_q2
                for ft in range(ND):
                    nc.tensor.matmul(
                        dst[:, ft, :], wrg[:, ft * P:(ft + 1) * P],
                        concat[:, ct, :],
                        start=False, stop=(ct % ND == ND - 1 and ct < ND))
            for ft in range(ND):  # += b_reg via K=1 matmuls; closes q2
                nc.tensor.matmul(
                    ps_q2[:, ft, :], breg_t[0:1, ft * P:(ft + 1) * P],
                    ones_row[0:1, 0:BL], start=False, stop=True)
            for ft in range(ND):
                nc.vector.scalar_tensor_tensor(
                    tmp_ep[:, ft, :], ps_ctx[:, ft, :], 1.0, rec_bc[:],
                    op0=ALU.mult, op1=ALU.mult)
                nc.vector.tensor_add(
                    out_sbT[:, ft, :], tmp_ep[:, ft, :], ps_q2[:, ft, :])
            for ft in range(ND):
                nc.sync.dma_start(out_d[ft * P:(ft + 1) * P, :],
                                  out_sbT[:, ft, :])
        kqbp_cm.__exit__(None, None, None)

    return _split_multiwaits(nc) if split else nc


_CACHED = {}


def _prep_in_maps(X, hidden_seq, W_ih, W_hh, b_ih, b_hh, W_q, b_q, W_k, b_k,
                  W_reg, b_reg):
    nf16, nf32 = np.float16, np.float32
    shared = {
        "wihT16": np.ascontiguousarray(W_ih.T).astype(nf16),
        "whhT16": np.ascontiguousarray(W_hh.T).astype(nf16),
        "wqT16": np.ascontiguousarray(W_q.T).astype(nf16),
        "wk16": np.ascontiguousarray(W_k).astype(nf16),
        "wregT16": np.ascontiguousarray(W_reg.T).astype(nf16),
        "bihh": (b_ih + b_hh).astype(nf32).reshape(D, 1),
        "bq": b_q.astype(nf32).reshape(D, 1),
        "breg1": b_reg.astype(nf16).reshape(1, F),
        "ident16": np.eye(P, dtype=nf16),
    }
    in_maps = []
    for c in range(NCORES):
        Xc = X[c * BL:(c + 1) * BL]                      # (4, 128, 1024)
        hsc = hidden_seq[c * BL:(c + 1) * BL]            # (4, 2048, 1024)
        hs16 = hsc.astype(nf16)
        m = dict(shared)
        m["xT16"] = np.ascontiguousarray(Xc.transpose(2, 1, 0).reshape(D, TB)).astype(nf16)
        m["hs16"] = hs16
        m["hsTh16"] = np.ascontiguousarray(hs16[:, NN * P:, :].transpose(0, 2, 1))
        m["h0T16"] = np.ascontiguousarray(hsc[:, -1, :].T).astype(nf16)
        in_maps.append(m)
    return in_maps


def kernel(**inputs):
    from concourse.bass_utils import run_bass_kernel_spmd

    if "nc" not in _CACHED:
        _CACHED["nc"] = build_program()
    nc = _CACHED["nc"]

    in_maps = _prep_in_maps(**inputs)
    core_ids = list(range(NCORES))
    res = run_bass_kernel_spmd(nc, in_maps, core_ids)
    outs = [res.results[i]["outT"].T for i in range(NCORES)]
    out = np.concatenate(outs, axis=0).astype(np.float32)
    return out.reshape(-1, 1, F)
